# revision 27
# baseline (speedup 1.0000x reference)
"""Bass/Tile kernel for nn_CausalSelfAttention (GQA + RMS-norm + RoPE + sliding window).

Sharding: 4-way sequence x 2-way heads over 8 NeuronCores.
Per core: 1024 queries, 8 q-heads, 2 kv-heads, kv buffer of 2048 rows
(1024-row halo to the left, zero-padded for the first sequence shard).

All layouts are transpose-free on device:
  - host passes x^T and W^T slices
  - projections produce q^T/k^T [hd, seq] (lhsT = W tile) and v [seq, hd]
    (lhsT = x^T tile) directly
  - scores^T [sk, sq] = k_tile^T.T @ q^T ; PV: y^T += v_tile.T @ probs^T
  - out^T = Wo^T.T @ y^T  (partial over this core's heads; host sums pairs)

RMS-norm scales fold into the RoPE multiply; 1/sqrt(hd) folds into the
q-side scale; softmax needs no running max (rms-normed logits bounded by
sqrt(128)). Causal/window edges are handled by multiplying probs with
host-precomputed 0/1 mask tiles on DVE (cheap, keeps Pool free); the
sliding-window structure means only 4 of 6 sk-tile pairs per query block
need a mask. Padded halo keys project to v=0 so they never pollute y;
only the softmax denominator needs the pz zeroing on the first shard.

Attention runs on 256-query blocks (10 sk tiles each instead of 12 per
512 thanks to the sliding window), emitted as a 2-head depth-2 software
pipeline: each step issues the NEXT pair's score matmuls before the
current pair's PV matmuls, so the PE never stalls on the
Act-exp -> DVE-mask chain.  Wo stays resident in SBUF; rms-norm
partition reduction runs on the PE (ones matmul) with two-stage drains
so the DVE tail never blocks the next matmul stream.
"""

import sys

if "/opt/trn_rl_repo" not in sys.path:
    sys.path.insert(0, "/opt/trn_rl_repo")

import ml_dtypes
import numpy as np

import concourse.mybir as mybir
import concourse.tile as tile
from concourse import bacc, bass_isa, bass_utils

f32 = mybir.dt.float32
f32r = mybir.dt.float32r
bf16 = mybir.dt.bfloat16
AF = mybir.ActivationFunctionType

D = 2048
S = 4096
NH = 16
NKV = 4
HD = 128
SEQW = 4
HEADW = 2
SQ = S // SEQW              # 1024 queries per core
HALO = 1024                 # local window
KVLEN = SQ + HALO           # 2048
QH = NH // HEADW            # 8 q-heads per core
KVH = NKV // HEADW          # 2 kv-heads per core
NB = 512                    # matmul moving block
NSQB = SQ // NB             # 2
NDT = D // 128              # 16
NKT = KVLEN // 128          # 16
EPS = 1.1920929e-07


def build_program():
    nc = bacc.Bacc(
        "TRN2",
        target_bir_lowering=False,
        debug=False,
        enable_asserts=False,
        num_devices=8,
    )
    xtkv = nc.dram_tensor("xtkv", [D, KVLEN], bf16, kind="ExternalInput").ap()
    wqt = nc.dram_tensor("wqt", [D, QH * HD], bf16, kind="ExternalInput").ap()
    wkt = nc.dram_tensor("wkt", [D, KVH * HD], bf16, kind="ExternalInput").ap()
    wvt = nc.dram_tensor("wvt", [D, KVH * HD], bf16, kind="ExternalInput").ap()
    wot = nc.dram_tensor("wot", [QH * HD, D], bf16, kind="ExternalInput").ap()
    ckd = nc.dram_tensor("ck", [128, KVLEN], bf16, kind="ExternalInput").ap()
    skd = nc.dram_tensor("sk", [128, KVLEN], bf16, kind="ExternalInput").ap()
    pzd = nc.dram_tensor("pz", [128, 1], f32, kind="ExternalInput").ap()
    mskd = nc.dram_tensor(
        "msk", [128, 8, 256], bf16, kind="ExternalInput"
    ).ap()
    outT = nc.dram_tensor("outT", [D, SQ], bf16, kind="ExternalOutput").ap()

    with tile.TileContext(nc) as tc:
        with (
            tc.tile_pool(name="persist", bufs=1) as persist,
            tc.tile_pool(name="scratch", bufs=5) as sc,
            tc.tile_pool(name="rows", bufs=3) as rows,
        ):
            # --- constants ---
            ones_col = persist.tile([128, 1], bf16)
            nc.vector.memset(ones_col, 1.0)
            ones128 = persist.tile([128, 128], bf16)
            nc.vector.memset(ones128, 1.0)
            eps_q = persist.tile([128, 1], f32)
            nc.vector.memset(eps_q, 128.0 * EPS)
            eps_k = persist.tile([128, 1], f32)
            nc.vector.memset(eps_k, EPS)
            pz_sb = persist.tile([128, 1], f32)

            ck_sb = persist.tile([128, KVLEN], bf16)
            sk_sb = persist.tile([128, KVLEN], bf16)
            wk_sb = persist.tile([128, NDT, KVH * HD], bf16)
            wv_sb = persist.tile([128, NDT, KVH * HD], bf16)
            msk_sb = persist.tile([128, 8, 256], bf16)
            wot_sb = persist.tile([128, QH, D], bf16)

            qrot = persist.tile([128, QH, SQ], bf16)
            krot = persist.tile([128, KVH, KVLEN], bf16)
            v_sb = persist.tile([128, NKT, KVH * HD], bf16)
            yt = persist.tile([128, QH, SQ], bf16)

            def drain_stage1(acc, s_scale, s_bias, nm, psA, artag="ar"):
                """acc: PSUM [128, NB] raw projection.  Short chain so the PE
                partition-reduce never waits long: raw copy (Act) -> square
                (DVE) -> ones128 matmul (PE) -> sqrt (Act)."""
                raw = sc.tile([128, NB], bf16, tag="big0", name=f"raw{nm}")
                nc.scalar.copy(out=raw, in_=acc)
                sqd_t = sc.tile([128, NB], bf16, tag="big1", name=f"sqd{nm}")
                nc.vector.tensor_mul(out=sqd_t, in0=raw, in1=raw)
                allr = psA.tile([128, NB], f32, tag=artag, name=f"allr{nm}")
                nc.tensor.matmul(
                    allr, lhsT=ones128, rhs=sqd_t, start=True, stop=True
                )
                s_full = sc.tile([128, NB], f32, tag="big3", name=f"sf{nm}")
                nc.scalar.activation(
                    out=s_full, in_=allr, func=AF.Sqrt, bias=s_bias, scale=s_scale
                )
                return raw, s_full

            def drain_stage2(st, out_slice, ctab, stab, nm):
                """Norm+rope tail; runs on DVE/Pool, overlapping the next
                matmul stream."""
                raw, s_full = st
                a_full = sc.tile([128, NB], bf16, tag="big4", name=f"af{nm}")
                with nc.allow_low_precision(reason="f32r is 4-byte fp32 storage"):
                    nc.vector.reciprocal(out=a_full, in_=s_full)
                rawa = sc.tile([128, NB], bf16, tag="big5", name=f"ra{nm}")
                nc.vector.tensor_mul(out=rawa, in0=raw, in1=a_full)
                t1 = sc.tile([128, NB], bf16, tag="big2", name=f"t1{nm}")
                nc.vector.tensor_mul(out=t1, in0=rawa, in1=ctab)
                t2 = sc.tile([128, NB], bf16, tag="big1", name=f"t2{nm}")
                nc.vector.tensor_mul(out=t2, in0=rawa, in1=stab)
                usw = sc.tile([128, NB], bf16, tag="big0", name=f"usw{nm}")
                nc.gpsimd.tensor_copy(out=usw[0:64, :], in_=t2[64:128, :])
                nc.gpsimd.tensor_copy(out=usw[64:128, :], in_=t2[0:64, :])
                nc.vector.tensor_add(
                    out=out_slice[0:64, :], in0=t1[0:64, :], in1=usw[0:64, :]
                )
                nc.vector.tensor_sub(
                    out=out_slice[64:128, :], in0=t1[64:128, :], in1=usw[64:128, :]
                )

            # ====== Projection phase: kv quarters + q blocks interleaved ======
            # Emission order qtr0,1,2, Qblk0, qtr3, Qblk1 so attention's
            # dependencies (krot/v then qrot) drain as early as possible.
            # Per quarter: k-stream (2 accs) then v-stream (4 accs) so k accs
            # complete and drain while v matmuls still run.  Q blocks reuse
            # the quarter's resident x tiles and run heads in 2 groups of 4.
            with tc.tile_pool(name="xs", bufs=6) as xs:
              with tc.tile_pool(name="psP", bufs=6, space="PSUM") as psP, \
                 tc.tile_pool(name="psA", bufs=2, space="PSUM") as psA:
                wk_r = wkt.rearrange("(c p) w -> p c w", p=128)
                wv_r = wvt.rearrange("(c p) w -> p c w", p=128)
                wot_r = wot.rearrange("(y p) d -> p y d", p=128)
                xts = {}

                def load_xtile(qtr, dc):
                    t = xs.tile(
                        [128, 4, NB], bf16, tag="xk", bufs=8, name=f"xk{qtr}_{dc}"
                    )
                    nc.sync.dma_start(
                        out=t,
                        in_=xtkv[
                            512 * dc : 512 * (dc + 1), NB * qtr : NB * (qtr + 1)
                        ].rearrange("(c p) w -> p c w", p=128),
                    )
                    xts[(qtr, dc)] = t

                def emit_qtr(qtr):
                    if qtr == 0:
                        # interleave weight/table loads with the first x tiles
                        for dc in range(4):
                            nc.sync.dma_start(
                                out=wk_sb[:, 4 * dc : 4 * (dc + 1), :],
                                in_=wk_r[:, 4 * dc : 4 * (dc + 1), :],
                            )
                            load_xtile(0, dc)
                            nc.sync.dma_start(
                                out=wv_sb[:, 4 * dc : 4 * (dc + 1), :],
                                in_=wv_r[:, 4 * dc : 4 * (dc + 1), :],
                            )
                        nc.sync.dma_start(out=ck_sb, in_=ckd)
                        nc.sync.dma_start(out=sk_sb, in_=skd)
                        nc.sync.dma_start(out=msk_sb, in_=mskd)
                        nc.sync.dma_start(out=pz_sb, in_=pzd)
                    else:
                        for dc in range(4):
                            load_xtile(qtr, dc)
                    nc.sync.dma_start(
                        out=wot_sb[:, :, 512 * qtr : 512 * (qtr + 1)],
                        in_=wot_r[:, :, 512 * qtr : 512 * (qtr + 1)],
                    )
                    kacc = {
                        kvh: psP.tile(
                            [128, NB], f32, tag="acc", name=f"kacc{qtr}_{kvh}"
                        )
                        for kvh in range(KVH)
                    }
                    for dc in range(4):
                        for dl in range(4):
                            d = 4 * dc + dl
                            for kvh in range(KVH):
                                nc.tensor.matmul(
                                    kacc[kvh],
                                    lhsT=wk_sb[:, d, HD * kvh : HD * (kvh + 1)],
                                    rhs=xts[(qtr, dc)][:, dl, :],
                                    start=(d == 0),
                                    stop=(d == NDT - 1),
                                )
                    kst = [
                        drain_stage1(
                            kacc[kvh], 1.0 / 128.0, eps_k, f"k{qtr}_{kvh}", psA
                        )
                        for kvh in range(KVH)
                    ]
                    for kvh in range(KVH):
                        drain_stage2(
                            kst[kvh],
                            krot[:, kvh, NB * qtr : NB * (qtr + 1)],
                            ck_sb[:, NB * qtr : NB * (qtr + 1)],
                            sk_sb[:, NB * qtr : NB * (qtr + 1)],
                            f"k{qtr}_{kvh}",
                        )
                    vacc = [
                        psP.tile(
                            [128, KVH * HD], f32, tag="acc", name=f"vacc{qtr}_{lt}"
                        )
                        for lt in range(4)
                    ]
                    for dc in range(4):
                        for dl in range(4):
                            d = 4 * dc + dl
                            for lt in range(4):
                                nc.tensor.matmul(
                                    vacc[lt],
                                    lhsT=xts[(qtr, dc)][:, dl, 128 * lt : 128 * (lt + 1)],
                                    rhs=wv_sb[:, d, :],
                                    start=(d == 0),
                                    stop=(d == NDT - 1),
                                )
                    for lt in range(4):
                        nc.scalar.copy(out=v_sb[:, 4 * qtr + lt, :], in_=vacc[lt])

                def emit_qblk(blk, hgs=(0, 1), accpool=None, acctag="acc",
                              arpool=None, artag="ar"):
                    qtr = 2 + blk
                    for hg in hgs:
                        acc = [
                            (accpool or psP).tile(
                                [128, NB], f32, tag=acctag,
                                name=f"qacc{blk}_{hg}_{hh}",
                            )
                            for hh in range(4)
                        ]
                        for dc in range(4):
                            wq2 = xs.tile(
                                [128, 4, 4 * HD], bf16, tag="wqd", bufs=3,
                                name=f"wqd{blk}_{hg}_{dc}",
                            )
                            nc.sync.dma_start(
                                out=wq2,
                                in_=wqt[
                                    512 * dc : 512 * (dc + 1),
                                    512 * hg : 512 * (hg + 1),
                                ].rearrange("(c p) w -> p c w", p=128),
                            )
                            for dl in range(4):
                                d = 4 * dc + dl
                                for hh in range(4):
                                    nc.tensor.matmul(
                                        acc[hh],
                                        lhsT=wq2[:, dl, HD * hh : HD * (hh + 1)],
                                        rhs=xts[(qtr, dc)][:, dl, :],
                                        start=(d == 0),
                                        stop=(d == NDT - 1),
                                    )
                        qst = [
                            drain_stage1(
                                acc[hh], 1.0, eps_q, f"q{blk}_{hg}_{hh}",
                                arpool or psA, artag,
                            )
                            for hh in range(4)
                        ]
                        for hh in range(4):
                            h = 4 * hg + hh
                            drain_stage2(
                                qst[hh],
                                qrot[:, h, NB * blk : NB * (blk + 1)],
                                ck_sb[:, HALO + NB * blk : HALO + NB * (blk + 1)],
                                sk_sb[:, HALO + NB * blk : HALO + NB * (blk + 1)],
                                f"q{blk}_{hg}_{hh}",
                            )

                emit_qtr(0)
                emit_qtr(1)
                emit_qtr(2)
                emit_qtr(3)
                emit_qblk(0)
                emit_qblk(1)

              # ============ Phase A + O: attention, then out-proj ============
              # 256-query attention blocks: each needs only 10 sk tiles
              # (vs 12 per 512-block) thanks to the sliding window.
              QB = 256
              with tc.tile_pool(name="probs", bufs=6) as pp, tc.tile_pool(
                name="psY", bufs=3, space="PSUM"
            ) as psY, tc.tile_pool(
                name="psR", bufs=1, space="PSUM"
            ) as psR, tc.tile_pool(
                name="psS", bufs=2, space="PSUM"
            ) as psS:
                for bo in range(NSQB):
                  for bi in range(2):
                    b = 2 * bo + bi
                    qsl = slice(QB * b, QB * (b + 1))
                    for kvh in range(KVH):
                        for hp in range(2):
                            h2 = (kvh * 4 + 2 * hp, kvh * 4 + 2 * hp + 1)
                            nmg = f"{b}_{kvh}_{hp}"
                            yacc = [
                                psY.tile([128, QB], f32, tag="y", name=f"y{nmg}_{a}")
                                for a in range(2)
                            ]
                            rsumA = pp.tile(
                                [128, QB], bf16, tag="rs", bufs=3,
                                name=f"rs{nmg}",
                            )
                            raccB = psR.tile(
                                [64, QB], f32, tag="r", name=f"rb{nmg}"
                            )

                            def sc4(j):
                                # both heads' scores for sk-tile pair j in one
                                # 2-bank PSUM tile: [jj0-A, jj0-B, jj1-A, jj1-B]
                                t = psS.tile(
                                    [128, 4, QB], f32, tag="s",
                                    name=f"sa{nmg}_{j}",
                                )
                                for jj in range(2):
                                    kt = 2 * b + 2 * j + jj
                                    for a in range(2):
                                        nc.tensor.matmul(
                                            t[:, 2 * jj + a, :],
                                            lhsT=krot[
                                                :, kvh, 128 * kt : 128 * (kt + 1)
                                            ],
                                            rhs=qrot[:, h2[a], qsl],
                                            start=True,
                                            stop=True,
                                        )
                                return t

                            def em4(j, t):
                                pt = pp.tile(
                                    [128, 4, QB], bf16, tag="pt", bufs=6,
                                    name=f"pt{nmg}_{j}",
                                )
                                nc.scalar.activation(
                                    out=pt, in_=t, func=AF.Exp, bias=0.0, scale=1.0
                                )
                                if j == 0:
                                    nc.vector.tensor_mul(
                                        out=pt, in0=pt, in1=msk_sb[:, 0:4, :]
                                    )
                                elif j == 4:
                                    nc.vector.tensor_mul(
                                        out=pt, in0=pt, in1=msk_sb[:, 4:8, :]
                                    )
                                elif j < 4 - b:
                                    # pair fully inside the padded halo on the
                                    # first shard (v=0 there; only the softmax
                                    # denominator needs zeroing)
                                    nc.vector.tensor_scalar_mul(
                                        out=pt, in0=pt, scalar1=pz_sb
                                    )
                                return pt

                            def pv4(j, pt):
                                for jj in range(2):
                                    tt = 2 * j + jj
                                    kt = 2 * b + tt
                                    for a in range(2):
                                        nc.tensor.matmul(
                                            yacc[a],
                                            lhsT=v_sb[
                                                :, kt, HD * kvh : HD * (kvh + 1)
                                            ],
                                            rhs=pt[:, 2 * jj + a, :],
                                            start=(tt == 0),
                                            stop=(tt == 9),
                                        )
                                # prob row-sums: head A partials on DVE,
                                # head B via PE ones-matmuls (engine balance)
                                for jj in range(2):
                                    tt = 2 * j + jj
                                    nc.tensor.matmul(
                                        raccB[0:1, :],
                                        lhsT=ones_col,
                                        rhs=pt[:, 2 * jj + 1, :],
                                        start=(tt == 0),
                                        stop=(tt == 9),
                                        skip_group_check=True,
                                    )
                                if j == 0:
                                    nc.vector.tensor_add(
                                        out=rsumA, in0=pt[:, 0, :], in1=pt[:, 2, :]
                                    )
                                else:
                                    tmp = pp.tile(
                                        [128, QB], bf16, tag="rt", bufs=3,
                                        name=f"rt{nmg}_{j}",
                                    )
                                    nc.vector.tensor_add(
                                        out=tmp, in0=pt[:, 0, :], in1=pt[:, 2, :]
                                    )
                                    nc.vector.tensor_add(
                                        out=rsumA, in0=rsumA, in1=tmp
                                    )

                            # depth-2 pipeline over the 5 sk-tile pairs
                            sacc, ptb = {}, {}
                            sacc[0] = sc4(0)
                            sacc[1] = sc4(1)
                            ptb[0] = em4(0, sacc.pop(0))
                            for j in range(5):
                                if j + 2 < 5:
                                    sacc[j + 2] = sc4(j + 2)
                                if j + 1 < 5:
                                    ptb[j + 1] = em4(j + 1, sacc.pop(j + 1))
                                pv4(j, ptb.pop(j))

                            # head B first: frees the psR slot for head A's
                            # partition-reduce matmul
                            rinvB = rows.tile(
                                [1, QB], f32r, tag="r1", name=f"rinv{nmg}_B"
                            )
                            with nc.allow_low_precision(reason="f32r 4-byte"):
                                nc.vector.reciprocal(
                                    out=rinvB, in_=raccB[0:1, :]
                                )
                            rbB = sc.tile(
                                [128, QB], f32r, tag="big5", name=f"rbB{nmg}"
                            )
                            nc.gpsimd.partition_broadcast(rbB, rinvB, channels=128)
                            nc.vector.tensor_mul(
                                out=yt[:, h2[1], qsl], in0=yacc[1], in1=rbB
                            )
                            rall = psR.tile(
                                [128, QB], f32, tag="r", name=f"rl{nmg}_A"
                            )
                            nc.tensor.matmul(
                                rall, lhsT=ones128, rhs=rsumA, start=True, stop=True
                            )
                            # ones128 already lands the sum on every partition,
                            # so reciprocal runs on the full tile (no broadcast)
                            rinvA = sc.tile(
                                [128, QB], f32r, tag="big5", name=f"rbA{nmg}"
                            )
                            with nc.allow_low_precision(reason="f32r 4-byte"):
                                nc.vector.reciprocal(out=rinvA, in_=rall)
                            nc.vector.tensor_mul(
                                out=yt[:, h2[0], qsl], in0=yacc[0], in1=rinvA
                            )
                  # ---- out-proj for this 512-block (weights resident) ----
                  for dm in range(NDT):
                        oacc = psS.tile(
                            [128, NB], f32, tag="s", name=f"oacc{dm}_{bo}"
                        )
                        for y in range(QH):
                            nc.tensor.matmul(
                                oacc,
                                lhsT=wot_sb[:, y, 128 * dm : 128 * (dm + 1)],
                                rhs=yt[:, y, NB * bo : NB * (bo + 1)],
                                start=(y == 0),
                                stop=(y == QH - 1),
                            )
                        ot = sc.tile([128, NB], bf16, tag="big0", name=f"ot{dm}_{bo}")
                        if bo == NSQB - 1 and dm == NDT - 1:
                            for hv in range(2):
                                hs = slice(256 * hv, 256 * (hv + 1))
                                nc.vector.tensor_copy(
                                    out=ot[:, hs], in_=oacc[:, hs]
                                )
                                nc.sync.dma_start(
                                    out=outT[
                                        128 * dm : 128 * (dm + 1),
                                        NB * bo + 256 * hv : NB * bo + 256 * (hv + 1),
                                    ],
                                    in_=ot[:, hs],
                                )
                        else:
                            nc.vector.tensor_copy(out=ot, in_=oacc)
                            nc.sync.dma_start(
                                out=outT[
                                    128 * dm : 128 * (dm + 1),
                                    NB * bo : NB * (bo + 1),
                                ],
                                in_=ot,
                            )

    nc.compile()
    return nc


def host_prep(x, Wq, Wk, Wv, Wo):
    x2 = np.asarray(x, dtype=np.float32).reshape(S, D)
    xT = np.ascontiguousarray(x2.T)
    WqT = np.ascontiguousarray(np.asarray(Wq, np.float32).T)
    WkT = np.ascontiguousarray(np.asarray(Wk, np.float32).T)
    WvT = np.ascontiguousarray(np.asarray(Wv, np.float32).T)
    WoT = np.ascontiguousarray(np.asarray(Wo, np.float32).T)

    pos = np.arange(-HALO, S, dtype=np.float32)
    invf = 1.0 / (10000.0 ** (np.arange(0, HD, 2, dtype=np.float32) / HD))
    fr = pos[:, None] * invf[None, :]
    cosT = np.cos(fr).T.astype(np.float32)
    sinT = np.sin(fr).T.astype(np.float32)
    C2 = np.ascontiguousarray(np.concatenate([cosT, cosT], axis=0))
    S2 = np.ascontiguousarray(np.concatenate([sinT, sinT], axis=0))

    in_maps = []
    for si in range(SEQW):
        lo = si * SQ - HALO
        xtkv = np.zeros((D, KVLEN), np.float32)
        lo_c = max(lo, 0)
        xtkv[:, lo_c - lo :] = xT[:, lo_c : si * SQ + SQ]
        cks = C2[:, HALO + lo : HALO + lo + KVLEN].astype(ml_dtypes.bfloat16)
        sks = S2[:, HALO + lo : HALO + lo + KVLEN].astype(ml_dtypes.bfloat16)
        pz = np.full((128, 1), 0.0 if si == 0 else 1.0, np.float32)

        # 0/1 prob masks for the window/causal edge sk-tile pairs, with the
        # first-shard halo baked in.  msk[p, 4*blk+im, jj*NB+c] corresponds to
        # key 128*(4*blk+2*ip+jj)+p and query 512*blk+c of this shard.
        # edge-tile prob masks for 256-query blocks: window tiles t=0,1 and
        # causal tiles t=8,9 (patterns are block-independent; first-shard
        # halo baked in).  Each tile appears twice, matching the fused
        # [jj0-headA, jj0-headB, jj1-headA, jj1-headB] prob layout.
        msk = np.zeros((128, 8, 256), np.float32)
        p_i = np.arange(128)
        c_i = np.arange(256)
        for im, t in enumerate((0, 0, 1, 1, 8, 8, 9, 9)):
            k_abs = si * SQ - HALO + 128 * t + p_i[:, None]
            q_abs = si * SQ + c_i[None, :]
            msk[:, im, :] = (
                (k_abs <= q_abs) & (k_abs > q_abs - HALO) & (k_abs >= 0)
            )
        msk_b = msk.astype(ml_dtypes.bfloat16)

        xtkv_b = xtkv.astype(ml_dtypes.bfloat16)
        for hi in range(HEADW):
            in_maps.append(
                dict(
                    xtkv=xtkv_b,
                    wqt=WqT[:, 1024 * hi : 1024 * (hi + 1)].astype(ml_dtypes.bfloat16),
                    wkt=WkT[:, 256 * hi : 256 * (hi + 1)].astype(ml_dtypes.bfloat16),
                    wvt=WvT[:, 256 * hi : 256 * (hi + 1)].astype(ml_dtypes.bfloat16),
                    wot=WoT[1024 * hi : 1024 * (hi + 1), :].astype(ml_dtypes.bfloat16),
                    ck=cks,
                    sk=sks,
                    pz=pz,
                    msk=msk_b,
                )
            )
    return in_maps


def host_post(results):
    out = np.empty((S, D), np.float32)
    for si in range(SEQW):
        acc = results[2 * si]["outT"].astype(np.float32) + results[
            2 * si + 1
        ]["outT"].astype(np.float32)
        out[si * SQ : (si + 1) * SQ, :] = acc.T
    return out.reshape(1, S, D)


_cached_nc = None


def get_nc():
    global _cached_nc
    if _cached_nc is None:
        _cached_nc = build_program()
    return _cached_nc


def kernel(**inputs):
    nc = get_nc()
    in_maps = host_prep(
        inputs["x"], inputs["Wq"], inputs["Wk"], inputs["Wv"], inputs["Wo"]
    )
    res = bass_utils.run_bass_kernel_spmd(nc, in_maps, core_ids=list(range(8)))
    return host_post(res.results)


# revision 28
# speedup vs baseline: 1.0025x; 1.0025x over previous
"""Bass/Tile kernel for nn_CausalSelfAttention (GQA + RMS-norm + RoPE + sliding window).

Sharding: 4-way sequence x 2-way heads over 8 NeuronCores.
Per core: 1024 queries, 8 q-heads, 2 kv-heads, kv buffer of 2048 rows
(1024-row halo to the left, zero-padded for the first sequence shard).

All layouts are transpose-free on device:
  - host passes x^T and W^T slices
  - projections produce q^T/k^T [hd, seq] (lhsT = W tile) and v [seq, hd]
    (lhsT = x^T tile) directly
  - scores^T [sk, sq] = k_tile^T.T @ q^T ; PV: y^T += v_tile.T @ probs^T
  - out^T = Wo^T.T @ y^T  (partial over this core's heads; host sums pairs)

RMS-norm scales fold into the RoPE multiply; 1/sqrt(hd) folds into the
q-side scale; softmax needs no running max (rms-normed logits bounded by
sqrt(128)). Causal/window edges are handled by multiplying probs with
host-precomputed 0/1 mask tiles on DVE (cheap, keeps Pool free); the
sliding-window structure means only 4 of 6 sk-tile pairs per query block
need a mask. Padded halo keys project to v=0 so they never pollute y;
only the softmax denominator needs the pz zeroing on the first shard.

Attention runs on 256-query blocks (10 sk tiles each instead of 12 per
512 thanks to the sliding window), emitted as a 2-head depth-2 software
pipeline: each step issues the NEXT pair's score matmuls before the
current pair's PV matmuls, so the PE never stalls on the
Act-exp -> DVE-mask chain.  Wo stays resident in SBUF; rms-norm
partition reduction runs on the PE (ones matmul) with two-stage drains
so the DVE tail never blocks the next matmul stream.
"""

import sys

if "/opt/trn_rl_repo" not in sys.path:
    sys.path.insert(0, "/opt/trn_rl_repo")

import ml_dtypes
import numpy as np

import concourse.mybir as mybir
import concourse.tile as tile
from concourse import bacc, bass_isa, bass_utils

f32 = mybir.dt.float32
f32r = mybir.dt.float32r
bf16 = mybir.dt.bfloat16
AF = mybir.ActivationFunctionType

D = 2048
S = 4096
NH = 16
NKV = 4
HD = 128
SEQW = 4
HEADW = 2
SQ = S // SEQW              # 1024 queries per core
HALO = 1024                 # local window
KVLEN = SQ + HALO           # 2048
QH = NH // HEADW            # 8 q-heads per core
KVH = NKV // HEADW          # 2 kv-heads per core
NB = 512                    # matmul moving block
NSQB = SQ // NB             # 2
NDT = D // 128              # 16
NKT = KVLEN // 128          # 16
EPS = 1.1920929e-07


def build_program():
    nc = bacc.Bacc(
        "TRN2",
        target_bir_lowering=False,
        debug=False,
        enable_asserts=False,
        num_devices=8,
    )
    xtkv = nc.dram_tensor("xtkv", [D, KVLEN], bf16, kind="ExternalInput").ap()
    wqt = nc.dram_tensor("wqt", [D, QH * HD], bf16, kind="ExternalInput").ap()
    wkt = nc.dram_tensor("wkt", [D, KVH * HD], bf16, kind="ExternalInput").ap()
    wvt = nc.dram_tensor("wvt", [D, KVH * HD], bf16, kind="ExternalInput").ap()
    wot = nc.dram_tensor("wot", [QH * HD, D], bf16, kind="ExternalInput").ap()
    ckd = nc.dram_tensor("ck", [128, KVLEN], bf16, kind="ExternalInput").ap()
    skd = nc.dram_tensor("sk", [128, KVLEN], bf16, kind="ExternalInput").ap()
    pzd = nc.dram_tensor("pz", [128, 1], f32, kind="ExternalInput").ap()
    mskd = nc.dram_tensor(
        "msk", [128, 8, 256], bf16, kind="ExternalInput"
    ).ap()
    outT = nc.dram_tensor("outT", [D, SQ], bf16, kind="ExternalOutput").ap()

    with tile.TileContext(nc) as tc:
        with (
            tc.tile_pool(name="persist", bufs=1) as persist,
            tc.tile_pool(name="scratch", bufs=5) as sc,
            tc.tile_pool(name="rows", bufs=3) as rows,
        ):
            # --- constants ---
            ones_col = persist.tile([128, 1], bf16)
            nc.vector.memset(ones_col, 1.0)
            ones128 = persist.tile([128, 128], bf16)
            nc.vector.memset(ones128, 1.0)
            eps_q = persist.tile([128, 1], f32)
            nc.vector.memset(eps_q, 128.0 * EPS)
            eps_k = persist.tile([128, 1], f32)
            nc.vector.memset(eps_k, EPS)
            pz_sb = persist.tile([128, 1], f32)

            ck_sb = persist.tile([128, KVLEN], bf16)
            sk_sb = persist.tile([128, KVLEN], bf16)
            wk_sb = persist.tile([128, NDT, KVH * HD], bf16)
            wv_sb = persist.tile([128, NDT, KVH * HD], bf16)
            msk_sb = persist.tile([128, 8, 256], bf16)
            wot_sb = persist.tile([128, QH, D], bf16)

            qrot = persist.tile([128, QH, SQ], bf16)
            krot = persist.tile([128, KVH, KVLEN], bf16)
            v_sb = persist.tile([128, NKT, KVH * HD], bf16)
            yt = persist.tile([128, QH, SQ], bf16)

            def drain_stage1(acc, s_scale, s_bias, nm, psA, artag="ar"):
                """acc: PSUM [128, NB] raw projection.  Short chain so the PE
                partition-reduce never waits long: raw copy (Act) -> square
                (DVE) -> ones128 matmul (PE) -> sqrt (Act)."""
                raw = sc.tile([128, NB], bf16, tag="big0", name=f"raw{nm}")
                nc.scalar.copy(out=raw, in_=acc)
                sqd_t = sc.tile([128, NB], bf16, tag="big1", name=f"sqd{nm}")
                nc.vector.tensor_mul(out=sqd_t, in0=raw, in1=raw)
                allr = psA.tile([128, NB], f32, tag=artag, name=f"allr{nm}")
                nc.tensor.matmul(
                    allr, lhsT=ones128, rhs=sqd_t, start=True, stop=True
                )
                s_full = sc.tile([128, NB], f32, tag="big3", name=f"sf{nm}")
                nc.scalar.activation(
                    out=s_full, in_=allr, func=AF.Sqrt, bias=s_bias, scale=s_scale
                )
                return raw, s_full

            def drain_stage2(st, out_slice, ctab, stab, nm):
                """Norm+rope tail; runs on DVE/Pool, overlapping the next
                matmul stream."""
                raw, s_full = st
                a_full = sc.tile([128, NB], bf16, tag="big4", name=f"af{nm}")
                with nc.allow_low_precision(reason="f32r is 4-byte fp32 storage"):
                    nc.vector.reciprocal(out=a_full, in_=s_full)
                rawa = sc.tile([128, NB], bf16, tag="big5", name=f"ra{nm}")
                nc.vector.tensor_mul(out=rawa, in0=raw, in1=a_full)
                t1 = sc.tile([128, NB], bf16, tag="big2", name=f"t1{nm}")
                nc.vector.tensor_mul(out=t1, in0=rawa, in1=ctab)
                t2 = sc.tile([128, NB], bf16, tag="big1", name=f"t2{nm}")
                nc.vector.tensor_mul(out=t2, in0=rawa, in1=stab)
                usw = sc.tile([128, NB], bf16, tag="big0", name=f"usw{nm}")
                nc.gpsimd.tensor_copy(out=usw[0:64, :], in_=t2[64:128, :])
                nc.gpsimd.tensor_copy(out=usw[64:128, :], in_=t2[0:64, :])
                nc.vector.tensor_add(
                    out=out_slice[0:64, :], in0=t1[0:64, :], in1=usw[0:64, :]
                )
                nc.vector.tensor_sub(
                    out=out_slice[64:128, :], in0=t1[64:128, :], in1=usw[64:128, :]
                )

            # ====== Projection phase: kv quarters + q blocks interleaved ======
            # Emission order qtr0,1,2, Qblk0, qtr3, Qblk1 so attention's
            # dependencies (krot/v then qrot) drain as early as possible.
            # Per quarter: k-stream (2 accs) then v-stream (4 accs) so k accs
            # complete and drain while v matmuls still run.  Q blocks reuse
            # the quarter's resident x tiles and run heads in 2 groups of 4.
            with tc.tile_pool(name="xs", bufs=6) as xs:
              with tc.tile_pool(name="psP", bufs=6, space="PSUM") as psP, \
                 tc.tile_pool(name="psA", bufs=2, space="PSUM") as psA:
                wk_r = wkt.rearrange("(c p) w -> p c w", p=128)
                wv_r = wvt.rearrange("(c p) w -> p c w", p=128)
                wot_r = wot.rearrange("(y p) d -> p y d", p=128)
                xts = {}

                def load_xtile(qtr, dc):
                    t = xs.tile(
                        [128, 4, NB], bf16, tag="xk", bufs=8, name=f"xk{qtr}_{dc}"
                    )
                    nc.sync.dma_start(
                        out=t,
                        in_=xtkv[
                            512 * dc : 512 * (dc + 1), NB * qtr : NB * (qtr + 1)
                        ].rearrange("(c p) w -> p c w", p=128),
                    )
                    xts[(qtr, dc)] = t

                def emit_qtr(qtr):
                    if qtr == 0:
                        # interleave weight/table loads with the first x tiles
                        for dc in range(4):
                            nc.sync.dma_start(
                                out=wk_sb[:, 4 * dc : 4 * (dc + 1), :],
                                in_=wk_r[:, 4 * dc : 4 * (dc + 1), :],
                            )
                            load_xtile(0, dc)
                            nc.sync.dma_start(
                                out=wv_sb[:, 4 * dc : 4 * (dc + 1), :],
                                in_=wv_r[:, 4 * dc : 4 * (dc + 1), :],
                            )
                        nc.sync.dma_start(out=ck_sb, in_=ckd)
                        nc.sync.dma_start(out=sk_sb, in_=skd)
                        nc.sync.dma_start(out=msk_sb, in_=mskd)
                        nc.sync.dma_start(out=pz_sb, in_=pzd)
                    else:
                        for dc in range(4):
                            load_xtile(qtr, dc)
                    nc.sync.dma_start(
                        out=wot_sb[:, :, 512 * qtr : 512 * (qtr + 1)],
                        in_=wot_r[:, :, 512 * qtr : 512 * (qtr + 1)],
                    )
                    kacc = {
                        kvh: psP.tile(
                            [128, NB], f32, tag="acc", name=f"kacc{qtr}_{kvh}"
                        )
                        for kvh in range(KVH)
                    }
                    for dc in range(4):
                        for dl in range(4):
                            d = 4 * dc + dl
                            for kvh in range(KVH):
                                nc.tensor.matmul(
                                    kacc[kvh],
                                    lhsT=wk_sb[:, d, HD * kvh : HD * (kvh + 1)],
                                    rhs=xts[(qtr, dc)][:, dl, :],
                                    start=(d == 0),
                                    stop=(d == NDT - 1),
                                )
                    kst = [
                        drain_stage1(
                            kacc[kvh], 1.0 / 128.0, eps_k, f"k{qtr}_{kvh}", psA
                        )
                        for kvh in range(KVH)
                    ]
                    for kvh in range(KVH):
                        drain_stage2(
                            kst[kvh],
                            krot[:, kvh, NB * qtr : NB * (qtr + 1)],
                            ck_sb[:, NB * qtr : NB * (qtr + 1)],
                            sk_sb[:, NB * qtr : NB * (qtr + 1)],
                            f"k{qtr}_{kvh}",
                        )
                    vacc = [
                        psP.tile(
                            [128, KVH * HD], f32, tag="acc", name=f"vacc{qtr}_{lt}"
                        )
                        for lt in range(4)
                    ]
                    for dc in range(4):
                        for dl in range(4):
                            d = 4 * dc + dl
                            for lt in range(4):
                                nc.tensor.matmul(
                                    vacc[lt],
                                    lhsT=xts[(qtr, dc)][:, dl, 128 * lt : 128 * (lt + 1)],
                                    rhs=wv_sb[:, d, :],
                                    start=(d == 0),
                                    stop=(d == NDT - 1),
                                )
                    for lt in range(4):
                        nc.scalar.copy(out=v_sb[:, 4 * qtr + lt, :], in_=vacc[lt])

                def emit_qblk(blk, hgs=(0, 1), accpool=None, acctag="acc",
                              arpool=None, artag="ar"):
                    qtr = 2 + blk
                    for hg in hgs:
                        acc = [
                            (accpool or psP).tile(
                                [128, NB], f32, tag=acctag,
                                name=f"qacc{blk}_{hg}_{hh}",
                            )
                            for hh in range(4)
                        ]
                        for dc in range(4):
                            wq2 = xs.tile(
                                [128, 4, 4 * HD], bf16, tag="wqd", bufs=3,
                                name=f"wqd{blk}_{hg}_{dc}",
                            )
                            nc.sync.dma_start(
                                out=wq2,
                                in_=wqt[
                                    512 * dc : 512 * (dc + 1),
                                    512 * hg : 512 * (hg + 1),
                                ].rearrange("(c p) w -> p c w", p=128),
                            )
                            for dl in range(4):
                                d = 4 * dc + dl
                                for hh in range(4):
                                    nc.tensor.matmul(
                                        acc[hh],
                                        lhsT=wq2[:, dl, HD * hh : HD * (hh + 1)],
                                        rhs=xts[(qtr, dc)][:, dl, :],
                                        start=(d == 0),
                                        stop=(d == NDT - 1),
                                    )
                        qst = [
                            drain_stage1(
                                acc[hh], 1.0, eps_q, f"q{blk}_{hg}_{hh}",
                                arpool or psA, artag,
                            )
                            for hh in range(4)
                        ]
                        for hh in range(4):
                            h = 4 * hg + hh
                            drain_stage2(
                                qst[hh],
                                qrot[:, h, NB * blk : NB * (blk + 1)],
                                ck_sb[:, HALO + NB * blk : HALO + NB * (blk + 1)],
                                sk_sb[:, HALO + NB * blk : HALO + NB * (blk + 1)],
                                f"q{blk}_{hg}_{hh}",
                            )

                emit_qtr(0)
                emit_qtr(1)
                emit_qtr(2)
                emit_qtr(3)
                emit_qblk(0)
                emit_qblk(1)

              # ============ Phase A + O: attention, then out-proj ============
              # 256-query attention blocks: each needs only 10 sk tiles
              # (vs 12 per 512-block) thanks to the sliding window.
              QB = 256
              with tc.tile_pool(name="probs", bufs=6) as pp, tc.tile_pool(
                name="psY", bufs=3, space="PSUM"
            ) as psY, tc.tile_pool(
                name="psR", bufs=1, space="PSUM"
            ) as psR, tc.tile_pool(
                name="psS", bufs=2, space="PSUM"
            ) as psS:
                for bo in range(NSQB):
                  for bi in range(2):
                    b = 2 * bo + bi
                    qsl = slice(QB * b, QB * (b + 1))
                    for kvh in range(KVH):
                        for hp in range(2):
                            h2 = (kvh * 4 + 2 * hp, kvh * 4 + 2 * hp + 1)
                            nmg = f"{b}_{kvh}_{hp}"
                            yacc = [
                                psY.tile([128, QB], f32, tag="y", name=f"y{nmg}_{a}")
                                for a in range(2)
                            ]
                            rsumA = pp.tile(
                                [128, QB], bf16, tag="rs", bufs=2,
                                name=f"rs{nmg}",
                            )
                            raccB = psR.tile(
                                [64, QB], f32, tag="r", name=f"rb{nmg}"
                            )

                            def sc4(j):
                                # both heads' scores for sk-tile pair j in one
                                # 2-bank PSUM tile: [jj0-A, jj0-B, jj1-A, jj1-B]
                                t = psS.tile(
                                    [128, 4, QB], f32, tag="s",
                                    name=f"sa{nmg}_{j}",
                                )
                                for jj in range(2):
                                    kt = 2 * b + 2 * j + jj
                                    for a in range(2):
                                        nc.tensor.matmul(
                                            t[:, 2 * jj + a, :],
                                            lhsT=krot[
                                                :, kvh, 128 * kt : 128 * (kt + 1)
                                            ],
                                            rhs=qrot[:, h2[a], qsl],
                                            start=True,
                                            stop=True,
                                        )
                                return t

                            def em4(j, t):
                                pt = pp.tile(
                                    [128, 4, QB], bf16, tag="pt", bufs=4,
                                    name=f"pt{nmg}_{j}",
                                )
                                nc.scalar.activation(
                                    out=pt, in_=t, func=AF.Exp, bias=0.0, scale=1.0
                                )
                                if j == 0:
                                    nc.vector.tensor_mul(
                                        out=pt, in0=pt, in1=msk_sb[:, 0:4, :]
                                    )
                                elif j == 4:
                                    nc.vector.tensor_mul(
                                        out=pt, in0=pt, in1=msk_sb[:, 4:8, :]
                                    )
                                elif j < 4 - b:
                                    # pair fully inside the padded halo on the
                                    # first shard (v=0 there; only the softmax
                                    # denominator needs zeroing)
                                    nc.vector.tensor_scalar_mul(
                                        out=pt, in0=pt, scalar1=pz_sb
                                    )
                                return pt

                            def pv4(j, pt):
                                for jj in range(2):
                                    tt = 2 * j + jj
                                    kt = 2 * b + tt
                                    for a in range(2):
                                        nc.tensor.matmul(
                                            yacc[a],
                                            lhsT=v_sb[
                                                :, kt, HD * kvh : HD * (kvh + 1)
                                            ],
                                            rhs=pt[:, 2 * jj + a, :],
                                            start=(tt == 0),
                                            stop=(tt == 9),
                                        )
                                # prob row-sums: head A partials on DVE,
                                # head B via PE ones-matmuls (engine balance)
                                for jj in range(2):
                                    tt = 2 * j + jj
                                    nc.tensor.matmul(
                                        raccB[0:1, :],
                                        lhsT=ones_col,
                                        rhs=pt[:, 2 * jj + 1, :],
                                        start=(tt == 0),
                                        stop=(tt == 9),
                                        skip_group_check=True,
                                    )
                                if j == 0:
                                    nc.vector.tensor_add(
                                        out=rsumA, in0=pt[:, 0, :], in1=pt[:, 2, :]
                                    )
                                else:
                                    tmp = pp.tile(
                                        [128, QB], bf16, tag="rt", bufs=2,
                                        name=f"rt{nmg}_{j}",
                                    )
                                    nc.vector.tensor_add(
                                        out=tmp, in0=pt[:, 0, :], in1=pt[:, 2, :]
                                    )
                                    nc.vector.tensor_add(
                                        out=rsumA, in0=rsumA, in1=tmp
                                    )

                            # depth-2 pipeline over the 5 sk-tile pairs
                            sacc, ptb = {}, {}
                            sacc[0] = sc4(0)
                            sacc[1] = sc4(1)
                            ptb[0] = em4(0, sacc.pop(0))
                            for j in range(5):
                                if j + 2 < 5:
                                    sacc[j + 2] = sc4(j + 2)
                                if j + 1 < 5:
                                    ptb[j + 1] = em4(j + 1, sacc.pop(j + 1))
                                pv4(j, ptb.pop(j))

                            # head B first: frees the psR slot for head A's
                            # partition-reduce matmul
                            rinvB = rows.tile(
                                [1, QB], f32r, tag="r1", name=f"rinv{nmg}_B"
                            )
                            with nc.allow_low_precision(reason="f32r 4-byte"):
                                nc.vector.reciprocal(
                                    out=rinvB, in_=raccB[0:1, :]
                                )
                            rbB = sc.tile(
                                [128, QB], f32r, tag="big5", name=f"rbB{nmg}"
                            )
                            nc.gpsimd.partition_broadcast(rbB, rinvB, channels=128)
                            nc.vector.tensor_mul(
                                out=yt[:, h2[1], qsl], in0=yacc[1], in1=rbB
                            )
                            rall = psR.tile(
                                [128, QB], f32, tag="r", name=f"rl{nmg}_A"
                            )
                            nc.tensor.matmul(
                                rall, lhsT=ones128, rhs=rsumA, start=True, stop=True
                            )
                            # ones128 already lands the sum on every partition,
                            # so reciprocal runs on the full tile (no broadcast)
                            rinvA = sc.tile(
                                [128, QB], f32r, tag="big5", name=f"rbA{nmg}"
                            )
                            with nc.allow_low_precision(reason="f32r 4-byte"):
                                nc.vector.reciprocal(out=rinvA, in_=rall)
                            nc.vector.tensor_mul(
                                out=yt[:, h2[0], qsl], in0=yacc[0], in1=rinvA
                            )
                  # ---- out-proj for this 512-block (weights resident) ----
                  for dm in range(NDT):
                        oacc = psS.tile(
                            [128, NB], f32, tag="s", name=f"oacc{dm}_{bo}"
                        )
                        for y in range(QH):
                            nc.tensor.matmul(
                                oacc,
                                lhsT=wot_sb[:, y, 128 * dm : 128 * (dm + 1)],
                                rhs=yt[:, y, NB * bo : NB * (bo + 1)],
                                start=(y == 0),
                                stop=(y == QH - 1),
                            )
                        ot = sc.tile([128, NB], bf16, tag="big0", name=f"ot{dm}_{bo}")
                        if bo == NSQB - 1 and dm == NDT - 1:
                            for hv in range(2):
                                hs = slice(256 * hv, 256 * (hv + 1))
                                nc.vector.tensor_copy(
                                    out=ot[:, hs], in_=oacc[:, hs]
                                )
                                nc.sync.dma_start(
                                    out=outT[
                                        128 * dm : 128 * (dm + 1),
                                        NB * bo + 256 * hv : NB * bo + 256 * (hv + 1),
                                    ],
                                    in_=ot[:, hs],
                                )
                        else:
                            nc.vector.tensor_copy(out=ot, in_=oacc)
                            nc.sync.dma_start(
                                out=outT[
                                    128 * dm : 128 * (dm + 1),
                                    NB * bo : NB * (bo + 1),
                                ],
                                in_=ot,
                            )

    nc.compile()
    return nc


def host_prep(x, Wq, Wk, Wv, Wo):
    x2 = np.asarray(x, dtype=np.float32).reshape(S, D)
    xT = np.ascontiguousarray(x2.T)
    WqT = np.ascontiguousarray(np.asarray(Wq, np.float32).T)
    WkT = np.ascontiguousarray(np.asarray(Wk, np.float32).T)
    WvT = np.ascontiguousarray(np.asarray(Wv, np.float32).T)
    WoT = np.ascontiguousarray(np.asarray(Wo, np.float32).T)

    pos = np.arange(-HALO, S, dtype=np.float32)
    invf = 1.0 / (10000.0 ** (np.arange(0, HD, 2, dtype=np.float32) / HD))
    fr = pos[:, None] * invf[None, :]
    cosT = np.cos(fr).T.astype(np.float32)
    sinT = np.sin(fr).T.astype(np.float32)
    C2 = np.ascontiguousarray(np.concatenate([cosT, cosT], axis=0))
    S2 = np.ascontiguousarray(np.concatenate([sinT, sinT], axis=0))

    in_maps = []
    for si in range(SEQW):
        lo = si * SQ - HALO
        xtkv = np.zeros((D, KVLEN), np.float32)
        lo_c = max(lo, 0)
        xtkv[:, lo_c - lo :] = xT[:, lo_c : si * SQ + SQ]
        cks = C2[:, HALO + lo : HALO + lo + KVLEN].astype(ml_dtypes.bfloat16)
        sks = S2[:, HALO + lo : HALO + lo + KVLEN].astype(ml_dtypes.bfloat16)
        pz = np.full((128, 1), 0.0 if si == 0 else 1.0, np.float32)

        # 0/1 prob masks for the window/causal edge sk-tile pairs, with the
        # first-shard halo baked in.  msk[p, 4*blk+im, jj*NB+c] corresponds to
        # key 128*(4*blk+2*ip+jj)+p and query 512*blk+c of this shard.
        # edge-tile prob masks for 256-query blocks: window tiles t=0,1 and
        # causal tiles t=8,9 (patterns are block-independent; first-shard
        # halo baked in).  Each tile appears twice, matching the fused
        # [jj0-headA, jj0-headB, jj1-headA, jj1-headB] prob layout.
        msk = np.zeros((128, 8, 256), np.float32)
        p_i = np.arange(128)
        c_i = np.arange(256)
        for im, t in enumerate((0, 0, 1, 1, 8, 8, 9, 9)):
            k_abs = si * SQ - HALO + 128 * t + p_i[:, None]
            q_abs = si * SQ + c_i[None, :]
            msk[:, im, :] = (
                (k_abs <= q_abs) & (k_abs > q_abs - HALO) & (k_abs >= 0)
            )
        msk_b = msk.astype(ml_dtypes.bfloat16)

        xtkv_b = xtkv.astype(ml_dtypes.bfloat16)
        for hi in range(HEADW):
            in_maps.append(
                dict(
                    xtkv=xtkv_b,
                    wqt=WqT[:, 1024 * hi : 1024 * (hi + 1)].astype(ml_dtypes.bfloat16),
                    wkt=WkT[:, 256 * hi : 256 * (hi + 1)].astype(ml_dtypes.bfloat16),
                    wvt=WvT[:, 256 * hi : 256 * (hi + 1)].astype(ml_dtypes.bfloat16),
                    wot=WoT[1024 * hi : 1024 * (hi + 1), :].astype(ml_dtypes.bfloat16),
                    ck=cks,
                    sk=sks,
                    pz=pz,
                    msk=msk_b,
                )
            )
    return in_maps


def host_post(results):
    out = np.empty((S, D), np.float32)
    for si in range(SEQW):
        acc = results[2 * si]["outT"].astype(np.float32) + results[
            2 * si + 1
        ]["outT"].astype(np.float32)
        out[si * SQ : (si + 1) * SQ, :] = acc.T
    return out.reshape(1, S, D)


_cached_nc = None


def get_nc():
    global _cached_nc
    if _cached_nc is None:
        _cached_nc = build_program()
    return _cached_nc


def kernel(**inputs):
    nc = get_nc()
    in_maps = host_prep(
        inputs["x"], inputs["Wq"], inputs["Wk"], inputs["Wv"], inputs["Wo"]
    )
    res = bass_utils.run_bass_kernel_spmd(nc, in_maps, core_ids=list(range(8)))
    return host_post(res.results)


# revision 29
# speedup vs baseline: 1.0026x; 1.0002x over previous
"""Bass/Tile kernel for nn_CausalSelfAttention (GQA + RMS-norm + RoPE + sliding window).

Sharding: 4-way sequence x 2-way heads over 8 NeuronCores.
Per core: 1024 queries, 8 q-heads, 2 kv-heads, kv buffer of 2048 rows
(1024-row halo to the left, zero-padded for the first sequence shard).

All layouts are transpose-free on device:
  - host passes x^T and W^T slices
  - projections produce q^T/k^T [hd, seq] (lhsT = W tile) and v [seq, hd]
    (lhsT = x^T tile) directly
  - scores^T [sk, sq] = k_tile^T.T @ q^T ; PV: y^T += v_tile.T @ probs^T
  - out^T = Wo^T.T @ y^T  (partial over this core's heads; host sums pairs)

RMS-norm scales fold into the RoPE multiply; 1/sqrt(hd) folds into the
q-side scale; softmax needs no running max (rms-normed logits bounded by
sqrt(128)). Causal/window edges are handled by multiplying probs with
host-precomputed 0/1 mask tiles on DVE (cheap, keeps Pool free); the
sliding-window structure means only 4 of 6 sk-tile pairs per query block
need a mask. Padded halo keys project to v=0 so they never pollute y;
only the softmax denominator needs the pz zeroing on the first shard.

Attention runs on 256-query blocks (10 sk tiles each instead of 12 per
512 thanks to the sliding window), emitted as a 2-head depth-2 software
pipeline: each step issues the NEXT pair's score matmuls before the
current pair's PV matmuls, so the PE never stalls on the
Act-exp -> DVE-mask chain.  Wo stays resident in SBUF; rms-norm
partition reduction runs on the PE (ones matmul) with two-stage drains
so the DVE tail never blocks the next matmul stream.
"""

import sys

if "/opt/trn_rl_repo" not in sys.path:
    sys.path.insert(0, "/opt/trn_rl_repo")

import ml_dtypes
import numpy as np

import concourse.mybir as mybir
import concourse.tile as tile
from concourse import bacc, bass_isa, bass_utils

f32 = mybir.dt.float32
f32r = mybir.dt.float32r
bf16 = mybir.dt.bfloat16
AF = mybir.ActivationFunctionType

D = 2048
S = 4096
NH = 16
NKV = 4
HD = 128
SEQW = 4
HEADW = 2
SQ = S // SEQW              # 1024 queries per core
HALO = 1024                 # local window
KVLEN = SQ + HALO           # 2048
QH = NH // HEADW            # 8 q-heads per core
KVH = NKV // HEADW          # 2 kv-heads per core
NB = 512                    # matmul moving block
NSQB = SQ // NB             # 2
NDT = D // 128              # 16
NKT = KVLEN // 128          # 16
EPS = 1.1920929e-07


def build_program():
    nc = bacc.Bacc(
        "TRN2",
        target_bir_lowering=False,
        debug=False,
        enable_asserts=False,
        num_devices=8,
    )
    xtkv = nc.dram_tensor("xtkv", [D, KVLEN], bf16, kind="ExternalInput").ap()
    wqt = nc.dram_tensor("wqt", [D, QH * HD], bf16, kind="ExternalInput").ap()
    wkt = nc.dram_tensor("wkt", [D, KVH * HD], bf16, kind="ExternalInput").ap()
    wvt = nc.dram_tensor("wvt", [D, KVH * HD], bf16, kind="ExternalInput").ap()
    wot = nc.dram_tensor("wot", [QH * HD, D], bf16, kind="ExternalInput").ap()
    ckd = nc.dram_tensor("ck", [128, KVLEN], bf16, kind="ExternalInput").ap()
    skd = nc.dram_tensor("sk", [128, KVLEN], bf16, kind="ExternalInput").ap()
    pzd = nc.dram_tensor("pz", [128, 1], f32, kind="ExternalInput").ap()
    mskd = nc.dram_tensor(
        "msk", [128, 8, 256], bf16, kind="ExternalInput"
    ).ap()
    outT = nc.dram_tensor("outT", [D, SQ], bf16, kind="ExternalOutput").ap()

    with tile.TileContext(nc) as tc:
        with (
            tc.tile_pool(name="persist", bufs=1) as persist,
            tc.tile_pool(name="scratch", bufs=5) as sc,
            tc.tile_pool(name="rows", bufs=3) as rows,
        ):
            # --- constants ---
            ones_col = persist.tile([128, 1], bf16)
            nc.vector.memset(ones_col, 1.0)
            ones128 = persist.tile([128, 128], bf16)
            nc.vector.memset(ones128, 1.0)
            eps_q = persist.tile([128, 1], f32)
            nc.vector.memset(eps_q, 128.0 * EPS)
            eps_k = persist.tile([128, 1], f32)
            nc.vector.memset(eps_k, EPS)
            pz_sb = persist.tile([128, 1], f32)

            ck_sb = persist.tile([128, KVLEN], bf16)
            sk_sb = persist.tile([128, KVLEN], bf16)
            wk_sb = persist.tile([128, NDT, KVH * HD], bf16)
            wv_sb = persist.tile([128, NDT, KVH * HD], bf16)
            msk_sb = persist.tile([128, 8, 256], bf16)
            wot_sb = persist.tile([128, QH, D], bf16)

            qrot = persist.tile([128, QH, SQ], bf16)
            krot = persist.tile([128, KVH, KVLEN], bf16)
            v_sb = persist.tile([128, NKT, KVH * HD], bf16)
            yt = persist.tile([128, QH, SQ], bf16)

            def drain_stage1(acc, s_scale, s_bias, nm, psA, artag="ar"):
                """acc: PSUM [128, NB] raw projection.  Short chain so the PE
                partition-reduce never waits long: raw copy (Act) -> square
                (DVE) -> ones128 matmul (PE) -> sqrt (Act)."""
                raw = sc.tile([128, NB], bf16, tag="big0", name=f"raw{nm}")
                nc.scalar.copy(out=raw, in_=acc)
                sqd_t = sc.tile([128, NB], bf16, tag="big1", name=f"sqd{nm}")
                nc.vector.tensor_mul(out=sqd_t, in0=raw, in1=raw)
                allr = psA.tile([128, NB], f32, tag=artag, name=f"allr{nm}")
                nc.tensor.matmul(
                    allr, lhsT=ones128, rhs=sqd_t, start=True, stop=True
                )
                s_full = sc.tile([128, NB], f32, tag="big3", name=f"sf{nm}")
                nc.scalar.activation(
                    out=s_full, in_=allr, func=AF.Sqrt, bias=s_bias, scale=s_scale
                )
                return raw, s_full

            def drain_stage2(st, out_slice, ctab, stab, nm):
                """Norm+rope tail; runs on DVE/Pool, overlapping the next
                matmul stream."""
                raw, s_full = st
                a_full = sc.tile([128, NB], bf16, tag="big4", name=f"af{nm}")
                with nc.allow_low_precision(reason="f32r is 4-byte fp32 storage"):
                    nc.vector.reciprocal(out=a_full, in_=s_full)
                rawa = sc.tile([128, NB], bf16, tag="big5", name=f"ra{nm}")
                nc.vector.tensor_mul(out=rawa, in0=raw, in1=a_full)
                t1 = sc.tile([128, NB], bf16, tag="big2", name=f"t1{nm}")
                nc.vector.tensor_mul(out=t1, in0=rawa, in1=ctab)
                t2 = sc.tile([128, NB], bf16, tag="big1", name=f"t2{nm}")
                nc.vector.tensor_mul(out=t2, in0=rawa, in1=stab)
                usw = sc.tile([128, NB], bf16, tag="big0", name=f"usw{nm}")
                nc.gpsimd.tensor_copy(out=usw[0:64, :], in_=t2[64:128, :])
                nc.gpsimd.tensor_copy(out=usw[64:128, :], in_=t2[0:64, :])
                nc.vector.tensor_add(
                    out=out_slice[0:64, :], in0=t1[0:64, :], in1=usw[0:64, :]
                )
                nc.vector.tensor_sub(
                    out=out_slice[64:128, :], in0=t1[64:128, :], in1=usw[64:128, :]
                )

            # ====== Projection phase: kv quarters + q blocks interleaved ======
            # Emission order qtr0,1,2, Qblk0, qtr3, Qblk1 so attention's
            # dependencies (krot/v then qrot) drain as early as possible.
            # Per quarter: k-stream (2 accs) then v-stream (4 accs) so k accs
            # complete and drain while v matmuls still run.  Q blocks reuse
            # the quarter's resident x tiles and run heads in 2 groups of 4.
            with tc.tile_pool(name="xs", bufs=6) as xs:
              with tc.tile_pool(name="psP", bufs=6, space="PSUM") as psP, \
                 tc.tile_pool(name="psA", bufs=2, space="PSUM") as psA:
                wk_r = wkt.rearrange("(c p) w -> p c w", p=128)
                wv_r = wvt.rearrange("(c p) w -> p c w", p=128)
                wot_r = wot.rearrange("(y p) d -> p y d", p=128)
                xts = {}

                def load_xtile(qtr, dc):
                    t = xs.tile(
                        [128, 4, NB], bf16, tag="xk", bufs=8, name=f"xk{qtr}_{dc}"
                    )
                    nc.sync.dma_start(
                        out=t,
                        in_=xtkv[
                            512 * dc : 512 * (dc + 1), NB * qtr : NB * (qtr + 1)
                        ].rearrange("(c p) w -> p c w", p=128),
                    )
                    xts[(qtr, dc)] = t

                def emit_qtr(qtr):
                    if qtr == 0:
                        # interleave weight/table loads with the first x tiles
                        for dc in range(4):
                            nc.sync.dma_start(
                                out=wk_sb[:, 4 * dc : 4 * (dc + 1), :],
                                in_=wk_r[:, 4 * dc : 4 * (dc + 1), :],
                            )
                            load_xtile(0, dc)
                        nc.sync.dma_start(out=wv_sb, in_=wv_r)
                        nc.sync.dma_start(out=ck_sb, in_=ckd)
                        nc.sync.dma_start(out=sk_sb, in_=skd)
                        nc.sync.dma_start(out=msk_sb, in_=mskd)
                        nc.sync.dma_start(out=pz_sb, in_=pzd)
                    else:
                        for dc in range(4):
                            load_xtile(qtr, dc)
                    nc.sync.dma_start(
                        out=wot_sb[:, :, 512 * qtr : 512 * (qtr + 1)],
                        in_=wot_r[:, :, 512 * qtr : 512 * (qtr + 1)],
                    )
                    kacc = {
                        kvh: psP.tile(
                            [128, NB], f32, tag="acc", name=f"kacc{qtr}_{kvh}"
                        )
                        for kvh in range(KVH)
                    }
                    for dc in range(4):
                        for dl in range(4):
                            d = 4 * dc + dl
                            for kvh in range(KVH):
                                nc.tensor.matmul(
                                    kacc[kvh],
                                    lhsT=wk_sb[:, d, HD * kvh : HD * (kvh + 1)],
                                    rhs=xts[(qtr, dc)][:, dl, :],
                                    start=(d == 0),
                                    stop=(d == NDT - 1),
                                )
                    kst = [
                        drain_stage1(
                            kacc[kvh], 1.0 / 128.0, eps_k, f"k{qtr}_{kvh}", psA
                        )
                        for kvh in range(KVH)
                    ]
                    for kvh in range(KVH):
                        drain_stage2(
                            kst[kvh],
                            krot[:, kvh, NB * qtr : NB * (qtr + 1)],
                            ck_sb[:, NB * qtr : NB * (qtr + 1)],
                            sk_sb[:, NB * qtr : NB * (qtr + 1)],
                            f"k{qtr}_{kvh}",
                        )
                    vacc = [
                        psP.tile(
                            [128, KVH * HD], f32, tag="acc", name=f"vacc{qtr}_{lt}"
                        )
                        for lt in range(4)
                    ]
                    for dc in range(4):
                        for dl in range(4):
                            d = 4 * dc + dl
                            for lt in range(4):
                                nc.tensor.matmul(
                                    vacc[lt],
                                    lhsT=xts[(qtr, dc)][:, dl, 128 * lt : 128 * (lt + 1)],
                                    rhs=wv_sb[:, d, :],
                                    start=(d == 0),
                                    stop=(d == NDT - 1),
                                )
                    for lt in range(4):
                        nc.scalar.copy(out=v_sb[:, 4 * qtr + lt, :], in_=vacc[lt])

                def emit_qblk(blk, hgs=(0, 1), accpool=None, acctag="acc",
                              arpool=None, artag="ar"):
                    qtr = 2 + blk
                    for hg in hgs:
                        acc = [
                            (accpool or psP).tile(
                                [128, NB], f32, tag=acctag,
                                name=f"qacc{blk}_{hg}_{hh}",
                            )
                            for hh in range(4)
                        ]
                        for dc in range(4):
                            wq2 = xs.tile(
                                [128, 4, 4 * HD], bf16, tag="wqd", bufs=3,
                                name=f"wqd{blk}_{hg}_{dc}",
                            )
                            nc.sync.dma_start(
                                out=wq2,
                                in_=wqt[
                                    512 * dc : 512 * (dc + 1),
                                    512 * hg : 512 * (hg + 1),
                                ].rearrange("(c p) w -> p c w", p=128),
                            )
                            for dl in range(4):
                                d = 4 * dc + dl
                                for hh in range(4):
                                    nc.tensor.matmul(
                                        acc[hh],
                                        lhsT=wq2[:, dl, HD * hh : HD * (hh + 1)],
                                        rhs=xts[(qtr, dc)][:, dl, :],
                                        start=(d == 0),
                                        stop=(d == NDT - 1),
                                    )
                        qst = [
                            drain_stage1(
                                acc[hh], 1.0, eps_q, f"q{blk}_{hg}_{hh}",
                                arpool or psA, artag,
                            )
                            for hh in range(4)
                        ]
                        for hh in range(4):
                            h = 4 * hg + hh
                            drain_stage2(
                                qst[hh],
                                qrot[:, h, NB * blk : NB * (blk + 1)],
                                ck_sb[:, HALO + NB * blk : HALO + NB * (blk + 1)],
                                sk_sb[:, HALO + NB * blk : HALO + NB * (blk + 1)],
                                f"q{blk}_{hg}_{hh}",
                            )

                emit_qtr(0)
                emit_qtr(1)
                emit_qtr(2)
                emit_qtr(3)
                emit_qblk(0)
                emit_qblk(1)

              # ============ Phase A + O: attention, then out-proj ============
              # 256-query attention blocks: each needs only 10 sk tiles
              # (vs 12 per 512-block) thanks to the sliding window.
              QB = 256
              with tc.tile_pool(name="probs", bufs=6) as pp, tc.tile_pool(
                name="psY", bufs=3, space="PSUM"
            ) as psY, tc.tile_pool(
                name="psR", bufs=1, space="PSUM"
            ) as psR, tc.tile_pool(
                name="psS", bufs=2, space="PSUM"
            ) as psS:
                for bo in range(NSQB):
                  for bi in range(2):
                    b = 2 * bo + bi
                    qsl = slice(QB * b, QB * (b + 1))
                    for kvh in range(KVH):
                        for hp in range(2):
                            h2 = (kvh * 4 + 2 * hp, kvh * 4 + 2 * hp + 1)
                            nmg = f"{b}_{kvh}_{hp}"
                            yacc = [
                                psY.tile([128, QB], f32, tag="y", name=f"y{nmg}_{a}")
                                for a in range(2)
                            ]
                            rsumA = pp.tile(
                                [128, QB], bf16, tag="rs", bufs=2,
                                name=f"rs{nmg}",
                            )
                            raccB = psR.tile(
                                [64, QB], f32, tag="r", name=f"rb{nmg}"
                            )

                            def sc4(j):
                                # both heads' scores for sk-tile pair j in one
                                # 2-bank PSUM tile: [jj0-A, jj0-B, jj1-A, jj1-B]
                                t = psS.tile(
                                    [128, 4, QB], f32, tag="s",
                                    name=f"sa{nmg}_{j}",
                                )
                                for jj in range(2):
                                    kt = 2 * b + 2 * j + jj
                                    for a in range(2):
                                        nc.tensor.matmul(
                                            t[:, 2 * jj + a, :],
                                            lhsT=krot[
                                                :, kvh, 128 * kt : 128 * (kt + 1)
                                            ],
                                            rhs=qrot[:, h2[a], qsl],
                                            start=True,
                                            stop=True,
                                        )
                                return t

                            def em4(j, t):
                                pt = pp.tile(
                                    [128, 4, QB], bf16, tag="pt", bufs=4,
                                    name=f"pt{nmg}_{j}",
                                )
                                nc.scalar.activation(
                                    out=pt, in_=t, func=AF.Exp, bias=0.0, scale=1.0
                                )
                                if j == 0:
                                    nc.vector.tensor_mul(
                                        out=pt, in0=pt, in1=msk_sb[:, 0:4, :]
                                    )
                                elif j == 4:
                                    nc.vector.tensor_mul(
                                        out=pt, in0=pt, in1=msk_sb[:, 4:8, :]
                                    )
                                elif j < 4 - b:
                                    # pair fully inside the padded halo on the
                                    # first shard (v=0 there; only the softmax
                                    # denominator needs zeroing)
                                    nc.vector.tensor_scalar_mul(
                                        out=pt, in0=pt, scalar1=pz_sb
                                    )
                                return pt

                            def pv4(j, pt):
                                for jj in range(2):
                                    tt = 2 * j + jj
                                    kt = 2 * b + tt
                                    for a in range(2):
                                        nc.tensor.matmul(
                                            yacc[a],
                                            lhsT=v_sb[
                                                :, kt, HD * kvh : HD * (kvh + 1)
                                            ],
                                            rhs=pt[:, 2 * jj + a, :],
                                            start=(tt == 0),
                                            stop=(tt == 9),
                                        )
                                # prob row-sums: head A partials on DVE,
                                # head B via PE ones-matmuls (engine balance)
                                for jj in range(2):
                                    tt = 2 * j + jj
                                    nc.tensor.matmul(
                                        raccB[0:1, :],
                                        lhsT=ones_col,
                                        rhs=pt[:, 2 * jj + 1, :],
                                        start=(tt == 0),
                                        stop=(tt == 9),
                                        skip_group_check=True,
                                    )
                                if j == 0:
                                    nc.vector.tensor_add(
                                        out=rsumA, in0=pt[:, 0, :], in1=pt[:, 2, :]
                                    )
                                else:
                                    tmp = pp.tile(
                                        [128, QB], bf16, tag="rt", bufs=2,
                                        name=f"rt{nmg}_{j}",
                                    )
                                    nc.vector.tensor_add(
                                        out=tmp, in0=pt[:, 0, :], in1=pt[:, 2, :]
                                    )
                                    nc.vector.tensor_add(
                                        out=rsumA, in0=rsumA, in1=tmp
                                    )

                            # depth-2 pipeline over the 5 sk-tile pairs
                            sacc, ptb = {}, {}
                            sacc[0] = sc4(0)
                            sacc[1] = sc4(1)
                            ptb[0] = em4(0, sacc.pop(0))
                            for j in range(5):
                                if j + 2 < 5:
                                    sacc[j + 2] = sc4(j + 2)
                                if j + 1 < 5:
                                    ptb[j + 1] = em4(j + 1, sacc.pop(j + 1))
                                pv4(j, ptb.pop(j))

                            # head B first: frees the psR slot for head A's
                            # partition-reduce matmul
                            rinvB = rows.tile(
                                [1, QB], f32r, tag="r1", name=f"rinv{nmg}_B"
                            )
                            with nc.allow_low_precision(reason="f32r 4-byte"):
                                nc.vector.reciprocal(
                                    out=rinvB, in_=raccB[0:1, :]
                                )
                            rbB = sc.tile(
                                [128, QB], f32r, tag="big5", name=f"rbB{nmg}"
                            )
                            nc.gpsimd.partition_broadcast(rbB, rinvB, channels=128)
                            nc.vector.tensor_mul(
                                out=yt[:, h2[1], qsl], in0=yacc[1], in1=rbB
                            )
                            rall = psR.tile(
                                [128, QB], f32, tag="r", name=f"rl{nmg}_A"
                            )
                            nc.tensor.matmul(
                                rall, lhsT=ones128, rhs=rsumA, start=True, stop=True
                            )
                            # ones128 already lands the sum on every partition,
                            # so reciprocal runs on the full tile (no broadcast)
                            rinvA = sc.tile(
                                [128, QB], f32r, tag="big5", name=f"rbA{nmg}"
                            )
                            with nc.allow_low_precision(reason="f32r 4-byte"):
                                nc.vector.reciprocal(out=rinvA, in_=rall)
                            nc.vector.tensor_mul(
                                out=yt[:, h2[0], qsl], in0=yacc[0], in1=rinvA
                            )
                  # ---- out-proj for this 512-block (weights resident) ----
                  for dm in range(NDT):
                        oacc = psS.tile(
                            [128, NB], f32, tag="s", name=f"oacc{dm}_{bo}"
                        )
                        for y in range(QH):
                            nc.tensor.matmul(
                                oacc,
                                lhsT=wot_sb[:, y, 128 * dm : 128 * (dm + 1)],
                                rhs=yt[:, y, NB * bo : NB * (bo + 1)],
                                start=(y == 0),
                                stop=(y == QH - 1),
                            )
                        ot = sc.tile([128, NB], bf16, tag="big0", name=f"ot{dm}_{bo}")
                        if bo == NSQB - 1 and dm == NDT - 1:
                            for hv in range(2):
                                hs = slice(256 * hv, 256 * (hv + 1))
                                nc.vector.tensor_copy(
                                    out=ot[:, hs], in_=oacc[:, hs]
                                )
                                nc.sync.dma_start(
                                    out=outT[
                                        128 * dm : 128 * (dm + 1),
                                        NB * bo + 256 * hv : NB * bo + 256 * (hv + 1),
                                    ],
                                    in_=ot[:, hs],
                                )
                        else:
                            nc.vector.tensor_copy(out=ot, in_=oacc)
                            nc.sync.dma_start(
                                out=outT[
                                    128 * dm : 128 * (dm + 1),
                                    NB * bo : NB * (bo + 1),
                                ],
                                in_=ot,
                            )

    nc.compile()
    return nc


def host_prep(x, Wq, Wk, Wv, Wo):
    x2 = np.asarray(x, dtype=np.float32).reshape(S, D)
    xT = np.ascontiguousarray(x2.T)
    WqT = np.ascontiguousarray(np.asarray(Wq, np.float32).T)
    WkT = np.ascontiguousarray(np.asarray(Wk, np.float32).T)
    WvT = np.ascontiguousarray(np.asarray(Wv, np.float32).T)
    WoT = np.ascontiguousarray(np.asarray(Wo, np.float32).T)

    pos = np.arange(-HALO, S, dtype=np.float32)
    invf = 1.0 / (10000.0 ** (np.arange(0, HD, 2, dtype=np.float32) / HD))
    fr = pos[:, None] * invf[None, :]
    cosT = np.cos(fr).T.astype(np.float32)
    sinT = np.sin(fr).T.astype(np.float32)
    C2 = np.ascontiguousarray(np.concatenate([cosT, cosT], axis=0))
    S2 = np.ascontiguousarray(np.concatenate([sinT, sinT], axis=0))

    in_maps = []
    for si in range(SEQW):
        lo = si * SQ - HALO
        xtkv = np.zeros((D, KVLEN), np.float32)
        lo_c = max(lo, 0)
        xtkv[:, lo_c - lo :] = xT[:, lo_c : si * SQ + SQ]
        cks = C2[:, HALO + lo : HALO + lo + KVLEN].astype(ml_dtypes.bfloat16)
        sks = S2[:, HALO + lo : HALO + lo + KVLEN].astype(ml_dtypes.bfloat16)
        pz = np.full((128, 1), 0.0 if si == 0 else 1.0, np.float32)

        # 0/1 prob masks for the window/causal edge sk-tile pairs, with the
        # first-shard halo baked in.  msk[p, 4*blk+im, jj*NB+c] corresponds to
        # key 128*(4*blk+2*ip+jj)+p and query 512*blk+c of this shard.
        # edge-tile prob masks for 256-query blocks: window tiles t=0,1 and
        # causal tiles t=8,9 (patterns are block-independent; first-shard
        # halo baked in).  Each tile appears twice, matching the fused
        # [jj0-headA, jj0-headB, jj1-headA, jj1-headB] prob layout.
        msk = np.zeros((128, 8, 256), np.float32)
        p_i = np.arange(128)
        c_i = np.arange(256)
        for im, t in enumerate((0, 0, 1, 1, 8, 8, 9, 9)):
            k_abs = si * SQ - HALO + 128 * t + p_i[:, None]
            q_abs = si * SQ + c_i[None, :]
            msk[:, im, :] = (
                (k_abs <= q_abs) & (k_abs > q_abs - HALO) & (k_abs >= 0)
            )
        msk_b = msk.astype(ml_dtypes.bfloat16)

        xtkv_b = xtkv.astype(ml_dtypes.bfloat16)
        for hi in range(HEADW):
            in_maps.append(
                dict(
                    xtkv=xtkv_b,
                    wqt=WqT[:, 1024 * hi : 1024 * (hi + 1)].astype(ml_dtypes.bfloat16),
                    wkt=WkT[:, 256 * hi : 256 * (hi + 1)].astype(ml_dtypes.bfloat16),
                    wvt=WvT[:, 256 * hi : 256 * (hi + 1)].astype(ml_dtypes.bfloat16),
                    wot=WoT[1024 * hi : 1024 * (hi + 1), :].astype(ml_dtypes.bfloat16),
                    ck=cks,
                    sk=sks,
                    pz=pz,
                    msk=msk_b,
                )
            )
    return in_maps


def host_post(results):
    out = np.empty((S, D), np.float32)
    for si in range(SEQW):
        acc = results[2 * si]["outT"].astype(np.float32) + results[
            2 * si + 1
        ]["outT"].astype(np.float32)
        out[si * SQ : (si + 1) * SQ, :] = acc.T
    return out.reshape(1, S, D)


_cached_nc = None


def get_nc():
    global _cached_nc
    if _cached_nc is None:
        _cached_nc = build_program()
    return _cached_nc


def kernel(**inputs):
    nc = get_nc()
    in_maps = host_prep(
        inputs["x"], inputs["Wq"], inputs["Wk"], inputs["Wv"], inputs["Wo"]
    )
    res = bass_utils.run_bass_kernel_spmd(nc, in_maps, core_ids=list(range(8)))
    return host_post(res.results)
